# revision 3
# baseline (speedup 1.0000x reference)
"""Trainium2 Bass kernel for nn_DensityLoss (retrieval kNN hinge loss).

Computes mean(relu(topk_smallest_dist(x_pred, x_target, k) - 1.0)).

Strategy (8 NeuronCores, SPMD, x_pred rows sharded):
  - Host sorts targets by ||b||^2 and lays them out so each of 1024
    "fold chunks" (strided positions {j + 1024k}) holds 16 targets of
    nearly equal ||b||^2. Host pre-transposes to [dim, n] (factor 2 of the
    cross term folded into a) and quantizes both sides to fp8e4m3.
  - Device per core: TensorE computes 2*a.b with fp8 DoubleRow matmuls
    (0.5 cycles/row, so the PE stays ahead of evacuation even at mid
    p-state; K=128 is packed as 2 k-tiles of 128 with the second a-tile
    zeroed and the b operand stride-0 broadcast). Per 128-row tile the
    32 matmuls land in eight [128,2048] fp32 PSUM tiles; ScalarE
    evacuates five of them (wide fp32->fp16 copies) while DVE consumes
    the other three with mixed max(slab, PSUM) ops (folding a slab each
    time) plus two all-fp16 2x-mode folds, leaving three [128,2048]
    fp16 tiles that DMA straight to host (output width 6144/row).
  - Host finishes the fold tree (6144 -> 1024 chunk maxima), adds the
    per-chunk -min||b||^2, picks the top-32 chunks per row (fp8
    screening noise puts a true top-5 chunk at worst rank ~12 on this
    distribution; 32 leaves a wide margin), rescores the 32*16 = 512
    candidates exactly in float64, takes top-k, hinges, averages.
"""

import numpy as np

N_CORES = 8
N_PRED = 8192
N_TGT = 16384
DIM = 128
ROWS_PER_CORE = N_PRED // N_CORES  # 1024
ROWTILES = ROWS_PER_CORE // 128    # 8
TILE_W = 2048                      # targets per PSUM tile (4 fp32 banks)
N_TILES = N_TGT // TILE_W          # 8 PSUM tiles per rowtile
OUT_TILES = 3                      # fp16 tiles shipped to host per rowtile
OUT_W = OUT_TILES * TILE_W         # 6144
FOLD_TO = 1024                     # chunk count (final, after host fold)
FOLD_S = N_TGT // FOLD_TO          # 16 targets per fold chunk
TOP_CHUNKS = 32
HINGE = 1.0

_CACHE = {}


def _build_nc():
    import concourse.bacc as bacc
    import concourse.bass as bass
    import concourse.mybir as mybir
    import concourse.tile as tile

    dt = mybir.dt
    nc = bacc.Bacc(
        "TRN2",
        target_bir_lowering=False,
        debug=False,
        num_devices=N_CORES,
    )
    # a_t: [dim, 2 k-tiles, rows]; k-tile 1 is zeros (DoubleRow packing).
    a_t = nc.dram_tensor("a_t", [DIM, 2, ROWS_PER_CORE], dt.float8e4, kind="ExternalInput")
    b_t = nc.dram_tensor("b_t", [DIM, N_TGT], dt.float8e4, kind="ExternalInput")
    cmx = nc.dram_tensor(
        "cmx", [ROWTILES, 128, OUT_W], dt.float16, kind="ExternalOutput"
    )

    DR = mybir.MatmulPerfMode.DoubleRow

    with tile.TileContext(nc) as tc:
        with (
            tc.tile_pool(name="const", bufs=1) as cpool,
            tc.tile_pool(name="psum", bufs=2, space="PSUM") as ppool,
            tc.tile_pool(name="slab", bufs=2) as spool,
        ):
            bt_sb = cpool.tile([DIM, N_TGT], dt.float8e4)
            at_sb = cpool.tile([DIM, 2, ROWS_PER_CORE], dt.float8e4)

            nc.sync.dma_start(out=at_sb[:], in_=a_t[:])
            # Fine-grained slices so the first matmuls start early.
            for s in range(N_TILES):
                sl = bass.ts(s, TILE_W)
                nc.sync.dma_start(out=bt_sb[:, sl], in_=b_t[:, sl])

            for rt in range(ROWTILES):
                lhsT = at_sb[:, :, bass.ts(rt, 128)]  # [128, 2, 128]

                def mk_psum(t, lhsT=lhsT):
                    ps = ppool.tile([128, TILE_W], dt.float32)
                    for j in range(TILE_W // 512):
                        rhs = bt_sb[:, bass.ts(t * (TILE_W // 512) + j, 512)]
                        nc.tensor.matmul(
                            ps[:, bass.ts(j, 512)],
                            lhsT,
                            rhs.unsqueeze(1).broadcast_to([DIM, 2, 512]),
                            start=True,
                            stop=True,
                            perf_mode=DR,
                        )
                    return ps

                def slab(tag):
                    return spool.tile([128, TILE_W], dt.float16, tag=tag, name=tag)

                # Tiles 0,2,4,6,7 -> ScalarE copies; 1,3,5 -> DVE mixed max.
                # DVE pairings fold the freshly copied slabs so only five
                # fp16 tiles remain, then two 2x-mode folds leave three.
                a0 = slab("a0")
                nc.scalar.copy(a0[:], mk_psum(0)[:])
                m0 = slab("m0")
                nc.vector.tensor_max(m0[:], a0[:], mk_psum(1)[:])
                a2 = slab("a2")
                nc.scalar.copy(a2[:], mk_psum(2)[:])
                m1 = slab("m1")
                nc.vector.tensor_max(m1[:], a2[:], mk_psum(3)[:])
                a4 = slab("a4")
                nc.scalar.copy(a4[:], mk_psum(4)[:])
                m2 = slab("m2")
                nc.vector.tensor_max(m2[:], a4[:], mk_psum(5)[:])
                a6 = slab("a6")
                nc.scalar.copy(a6[:], mk_psum(6)[:])
                a7 = slab("a7")
                nc.scalar.copy(a7[:], mk_psum(7)[:])
                f0 = slab("f0")
                nc.vector.tensor_max(f0[:], m0[:], m1[:])
                f1 = slab("f1")
                nc.vector.tensor_max(f1[:], m2[:], a6[:])
                nc.sync.dma_start(out=cmx[rt][:, 0:TILE_W], in_=f0[:])
                nc.sync.dma_start(out=cmx[rt][:, TILE_W : 2 * TILE_W], in_=f1[:])
                nc.sync.dma_start(out=cmx[rt][:, 2 * TILE_W : 3 * TILE_W], in_=a7[:])

    nc.compile()
    return nc


def _get_nc():
    if "nc" not in _CACHE:
        _CACHE["nc"] = _build_nc()
    return _CACHE["nc"]


def _prep(x_pred, x_target):
    """Host-side layout: sort targets by b2, stride into fold chunks."""
    import ml_dtypes

    b2 = np.einsum("ij,ij->i", x_target.astype(np.float64), x_target.astype(np.float64))
    order = np.argsort(b2, kind="stable")
    # position j + 1024*k holds the target of sorted rank 16*j + k
    perm = np.empty(N_TGT, np.int64)
    jj, kk = np.meshgrid(np.arange(FOLD_TO), np.arange(FOLD_S), indexing="ij")
    perm[jj + FOLD_TO * kk] = order[FOLD_S * jj + kk]

    a_t = np.zeros((DIM, 2, N_PRED), ml_dtypes.float8_e4m3)
    a_t[:, 0, :] = (2.0 * x_pred.T).astype(ml_dtypes.float8_e4m3)
    b_t = np.ascontiguousarray(x_target[perm].T.astype(ml_dtypes.float8_e4m3))
    nb2c_row = (-b2[order[::FOLD_S]]).astype(np.float32)  # -min b2 per chunk
    cand_map = order.reshape(FOLD_TO, FOLD_S)  # chunk j -> target ids
    return a_t, b_t, nb2c_row, cand_map


def _host_finish(x_pred, x_target, f1, nb2c_row, cand_map, k):
    """f1: [N_PRED, OUT_W] fp32 slab maxima; fold to chunk maxima, screen,
    rescore the top chunks exactly in float64."""
    n = x_pred.shape[0]
    f1 = f1.reshape(n, OUT_TILES, TILE_W).max(axis=1)
    f1 = np.maximum(f1[:, :FOLD_TO], f1[:, FOLD_TO:])
    chunk_val = f1 + nb2c_row
    ch = np.argpartition(-chunk_val, TOP_CHUNKS, axis=1)[:, :TOP_CHUNKS]
    tid = cand_map[ch].reshape(n, TOP_CHUNKS * FOLD_S)

    a64 = x_pred.astype(np.float64)
    b64 = x_target.astype(np.float64)
    a2 = np.einsum("ij,ij->i", a64, a64)
    b2 = np.einsum("ij,ij->i", b64, b64)

    vals = np.empty((n, k))
    B = 1024
    for s in range(0, n, B):
        t = tid[s : s + B]
        bg = b64[t]  # [B, C, DIM]
        dots = np.einsum("rd,rcd->rc", a64[s : s + B], bg, optimize=True)
        d2 = a2[s : s + B, None] + b2[t] - 2.0 * dots
        vals[s : s + B] = np.partition(d2, k - 1, axis=1)[:, :k]
    d = np.sqrt(np.maximum(vals, 0.0))
    return np.float32(np.maximum(d - HINGE, 0.0).mean(dtype=np.float64))


def _host_exact(x_pred, x_target, k):
    """Exact fallback (never expected in practice)."""
    a = x_pred.astype(np.float32)
    b = x_target.astype(np.float32)
    a2 = np.sum(a * a, axis=1)[:, None]
    b2 = np.sum(b * b, axis=1)[None, :]
    out = np.empty((a.shape[0], k), np.float64)
    B = 1024
    for s in range(0, a.shape[0], B):
        d2 = a2[s : s + B] + b2 - 2.0 * (a[s : s + B] @ b.T)
        out[s : s + B] = np.partition(d2, k - 1, axis=1)[:, :k].astype(np.float64)
    d = np.sqrt(np.maximum(out, 0.0))
    return np.float32(np.maximum(d - HINGE, 0.0).mean(dtype=np.float64))


def kernel(x_pred, x_target, top_k=5, _want_results=False):
    from concourse.bass_utils import run_bass_kernel_spmd

    x_pred = np.asarray(x_pred, dtype=np.float32)
    x_target = np.asarray(x_target, dtype=np.float32)
    k = int(top_k)
    if (
        k > TOP_CHUNKS
        or x_pred.shape != (N_PRED, DIM)
        or x_target.shape != (N_TGT, DIM)
    ):
        return _host_exact(x_pred, x_target, k)

    nc = _get_nc()
    a_t_full, b_t, nb2c_row, cand_map = _prep(x_pred, x_target)

    in_maps = []
    for c in range(N_CORES):
        in_maps.append(
            {
                "a_t": np.ascontiguousarray(
                    a_t_full[:, :, c * ROWS_PER_CORE : (c + 1) * ROWS_PER_CORE]
                ),
                "b_t": b_t,
            }
        )

    res = run_bass_kernel_spmd(nc, in_maps, list(range(N_CORES)))
    f1 = np.concatenate(
        [
            res.results[c]["cmx"].reshape(ROWS_PER_CORE, OUT_W)
            for c in range(N_CORES)
        ],
        axis=0,
    ).astype(np.float32)
    out = _host_finish(x_pred, x_target, f1, nb2c_row, cand_map, k)
    if _want_results:
        return out, res
    return out


# revision 5
# speedup vs baseline: 1.3419x; 1.3419x over previous
"""Trainium2 Bass kernel for nn_DensityLoss (retrieval kNN hinge loss).

Computes mean(relu(topk_smallest_dist(x_pred, x_target, k) - 1.0)).

Strategy (8 NeuronCores, SPMD, x_pred rows sharded):
  - Host sorts targets by ||b||^2 and lays them out so each of 1024
    "fold chunks" (strided positions {j + 1024k}) holds 16 targets of
    nearly equal ||b||^2. Host pre-transposes to [dim, n] (factor 2 of the
    cross term folded into a) and quantizes both sides to fp8e4m3.
  - Device per core: TensorE computes 2*a.b with fp8 DoubleRow matmuls
    (issues every ~216ns for 512 targets vs ~427ns for bf16; K=128 is
    packed as 2 k-tiles with the second a-tile zeroed and the b operand
    stride-0 broadcast). PSUM is divided into four [128,1024] tiles so
    each consumer always has a prefilled tile waiting (the two-buffer
    2048-wide variant stalls every op on a refill). Per 128-row tile
    the 16 PSUM lanes are drained by ScalarE (9 wide fp32->fp16 copies)
    and DVE (7 mixed max(slab,PSUM) ops chained through 2 scratch
    lanes, seeded from the first ScalarE slab, plus one 2048-wide
    all-fp16 2x-mode fold), leaving 7168 fp16 values per row that DMA
    to the host.
  - Host finishes the fold tree (7168 -> 1024 chunk maxima), adds the
    per-chunk -min||b||^2, picks the top-32 chunks per row (fp8
    screening noise puts a true top-5 chunk at worst rank ~12 on this
    distribution; 32 leaves a wide margin), rescores the 32*16 = 512
    candidates exactly in float64, takes top-k, hinges, averages.
"""

import numpy as np

N_CORES = 8
N_PRED = 8192
N_TGT = 16384
DIM = 128
ROWS_PER_CORE = N_PRED // N_CORES  # 1024
ROWTILES = ROWS_PER_CORE // 128    # 8
LANE = 1024                        # targets per PSUM tile (2 fp32 banks)
N_LANES = N_TGT // LANE            # 16 PSUM lanes per rowtile
OUT_W = 7168                       # fp16 values per row shipped to host
FOLD_TO = 1024                     # chunk count (final, after host fold)
FOLD_S = N_TGT // FOLD_TO          # 16 targets per fold chunk
TOP_CHUNKS = 32
HINGE = 1.0

_CACHE = {}


def _build_nc():
    import concourse.bacc as bacc
    import concourse.bass as bass
    import concourse.mybir as mybir
    import concourse.tile as tile

    dt = mybir.dt
    nc = bacc.Bacc(
        "TRN2",
        target_bir_lowering=False,
        debug=False,
        num_devices=N_CORES,
    )
    # a_t: [dim, 2 k-tiles, rows]; k-tile 1 is zeros (DoubleRow packing).
    a_t = nc.dram_tensor("a_t", [DIM, 2, ROWS_PER_CORE], dt.float8e4, kind="ExternalInput")
    b_t = nc.dram_tensor("b_t", [DIM, N_TGT], dt.float8e4, kind="ExternalInput")
    cmx = nc.dram_tensor(
        "cmx", [ROWTILES, 128, OUT_W], dt.float16, kind="ExternalOutput"
    )

    DR = mybir.MatmulPerfMode.DoubleRow

    # Slab layout per rowtile (fp16, per-partition offsets in elements):
    #   s0..s8   at [0 : 9216)     ScalarE copies (s0 seeds the DVE chain)
    #   t_ping   at [9216 : 10240) DVE chain scratch
    #   t_pong   at [10240: 11264) DVE chain scratch (chain of 7 ends here)
    #   f        at [11264: 13312) 2048-wide fold of s1..s4
    # Out pieces: f (2048) + s5..s8 (4096) + chain (1024) = 7168.
    SLAB_W = 13312
    T_PING, T_PONG, F_OFF = 9216, 10240, 11264

    with tile.TileContext(nc) as tc:
        with (
            tc.tile_pool(name="const", bufs=1) as cpool,
            tc.tile_pool(name="psum", bufs=4, space="PSUM") as ppool,
            tc.tile_pool(name="slab", bufs=2) as spool,
        ):
            bt_sb = cpool.tile([DIM, N_TGT], dt.float8e4)
            at_sb = cpool.tile([DIM, 2, ROWS_PER_CORE], dt.float8e4)

            nc.sync.dma_start(out=at_sb[:], in_=a_t[:])
            # Fine-grained slices so the first matmuls start early.
            for s in range(N_LANES):
                sl = bass.ts(s, LANE)
                nc.sync.dma_start(out=bt_sb[:, sl], in_=b_t[:, sl])

            for rt in range(ROWTILES):
                lhsT = at_sb[:, :, bass.ts(rt, 128)]  # [128, 2, 128]
                slab = spool.tile([128, SLAB_W], dt.float16)

                def mk_psum(lane, lhsT=lhsT):
                    ps = ppool.tile([128, LANE], dt.float32)
                    for j in range(LANE // 512):
                        rhs = bt_sb[:, bass.ts(lane * (LANE // 512) + j, 512)]
                        nc.tensor.matmul(
                            ps[:, bass.ts(j, 512)],
                            lhsT,
                            rhs.unsqueeze(1).broadcast_to([DIM, 2, 512]),
                            start=True,
                            stop=True,
                            perf_mode=DR,
                        )
                    return ps

                # Lanes 0,2,4,..,12,14,15 -> ScalarE slabs s0..s8;
                # lanes 1,3,..,13 -> DVE chain (seeded from s0).
                n_s = 0
                n_d = 0
                chain = None
                for lane in range(N_LANES):
                    if (lane % 2 == 0 and lane < 14) or lane >= 14:
                        dst = slab[:, n_s * LANE : (n_s + 1) * LANE]
                        nc.scalar.copy(dst, mk_psum(lane)[:])
                        n_s += 1
                    else:
                        src = slab[:, 0:LANE] if chain is None else chain
                        # alternate scratch lanes to avoid in-place ops
                        dst_off = T_PING if n_d % 2 == 0 else T_PONG
                        nxt = slab[:, dst_off : dst_off + LANE]
                        nc.vector.tensor_max(nxt, src, mk_psum(lane)[:])
                        chain = nxt
                        n_d += 1
                # 2048-wide fp16 fold of s1..s4 (2x mode)
                nc.vector.tensor_max(
                    slab[:, F_OFF : F_OFF + 2048],
                    slab[:, LANE : 3 * LANE],
                    slab[:, 3 * LANE : 5 * LANE],
                )
                nc.sync.dma_start(
                    out=cmx[rt][:, 0:2048], in_=slab[:, F_OFF : F_OFF + 2048]
                )
                nc.sync.dma_start(
                    out=cmx[rt][:, 2048:6144], in_=slab[:, 5 * LANE : 9 * LANE]
                )
                nc.sync.dma_start(out=cmx[rt][:, 6144:7168], in_=chain)

    nc.compile()
    return nc


def _get_nc():
    if "nc" not in _CACHE:
        _CACHE["nc"] = _build_nc()
    return _CACHE["nc"]


def _prep(x_pred, x_target):
    """Host-side layout: sort targets by b2, stride into fold chunks."""
    import ml_dtypes

    b2 = np.einsum("ij,ij->i", x_target.astype(np.float64), x_target.astype(np.float64))
    order = np.argsort(b2, kind="stable")
    # position j + 1024*k holds the target of sorted rank 16*j + k
    perm = np.empty(N_TGT, np.int64)
    jj, kk = np.meshgrid(np.arange(FOLD_TO), np.arange(FOLD_S), indexing="ij")
    perm[jj + FOLD_TO * kk] = order[FOLD_S * jj + kk]

    a_t = np.zeros((DIM, 2, N_PRED), ml_dtypes.float8_e4m3)
    a_t[:, 0, :] = (2.0 * x_pred.T).astype(ml_dtypes.float8_e4m3)
    b_t = np.ascontiguousarray(x_target[perm].T.astype(ml_dtypes.float8_e4m3))
    nb2c_row = (-b2[order[::FOLD_S]]).astype(np.float32)  # -min b2 per chunk
    cand_map = order.reshape(FOLD_TO, FOLD_S)  # chunk j -> target ids
    return a_t, b_t, nb2c_row, cand_map


def _host_finish(x_pred, x_target, f1, nb2c_row, cand_map, k):
    """f1: [N_PRED, OUT_W] fp32 slab maxima; fold to chunk maxima, screen,
    rescore the top chunks exactly in float64."""
    n = x_pred.shape[0]
    f1 = f1.reshape(n, OUT_W // FOLD_TO, FOLD_TO).max(axis=1)
    chunk_val = f1 + nb2c_row
    ch = np.argpartition(-chunk_val, TOP_CHUNKS, axis=1)[:, :TOP_CHUNKS]
    tid = cand_map[ch].reshape(n, TOP_CHUNKS * FOLD_S)

    a64 = x_pred.astype(np.float64)
    b64 = x_target.astype(np.float64)
    a2 = np.einsum("ij,ij->i", a64, a64)
    b2 = np.einsum("ij,ij->i", b64, b64)

    vals = np.empty((n, k))
    B = 1024
    for s in range(0, n, B):
        t = tid[s : s + B]
        bg = b64[t]  # [B, C, DIM]
        dots = np.einsum("rd,rcd->rc", a64[s : s + B], bg, optimize=True)
        d2 = a2[s : s + B, None] + b2[t] - 2.0 * dots
        vals[s : s + B] = np.partition(d2, k - 1, axis=1)[:, :k]
    d = np.sqrt(np.maximum(vals, 0.0))
    return np.float32(np.maximum(d - HINGE, 0.0).mean(dtype=np.float64))


def _host_exact(x_pred, x_target, k):
    """Exact fallback (never expected in practice)."""
    a = x_pred.astype(np.float32)
    b = x_target.astype(np.float32)
    a2 = np.sum(a * a, axis=1)[:, None]
    b2 = np.sum(b * b, axis=1)[None, :]
    out = np.empty((a.shape[0], k), np.float64)
    B = 1024
    for s in range(0, a.shape[0], B):
        d2 = a2[s : s + B] + b2 - 2.0 * (a[s : s + B] @ b.T)
        out[s : s + B] = np.partition(d2, k - 1, axis=1)[:, :k].astype(np.float64)
    d = np.sqrt(np.maximum(out, 0.0))
    return np.float32(np.maximum(d - HINGE, 0.0).mean(dtype=np.float64))


def kernel(x_pred, x_target, top_k=5, _want_results=False):
    from concourse.bass_utils import run_bass_kernel_spmd

    x_pred = np.asarray(x_pred, dtype=np.float32)
    x_target = np.asarray(x_target, dtype=np.float32)
    k = int(top_k)
    if (
        k > TOP_CHUNKS
        or x_pred.shape != (N_PRED, DIM)
        or x_target.shape != (N_TGT, DIM)
    ):
        return _host_exact(x_pred, x_target, k)

    nc = _get_nc()
    a_t_full, b_t, nb2c_row, cand_map = _prep(x_pred, x_target)

    in_maps = []
    for c in range(N_CORES):
        in_maps.append(
            {
                "a_t": np.ascontiguousarray(
                    a_t_full[:, :, c * ROWS_PER_CORE : (c + 1) * ROWS_PER_CORE]
                ),
                "b_t": b_t,
            }
        )

    res = run_bass_kernel_spmd(nc, in_maps, list(range(N_CORES)))
    f1 = np.concatenate(
        [
            res.results[c]["cmx"].reshape(ROWS_PER_CORE, OUT_W)
            for c in range(N_CORES)
        ],
        axis=0,
    ).astype(np.float32)
    out = _host_finish(x_pred, x_target, f1, nb2c_row, cand_map, k)
    if _want_results:
        return out, res
    return out


# revision 8
# speedup vs baseline: 1.3623x; 1.0152x over previous
"""Trainium2 Bass kernel for nn_DensityLoss (retrieval kNN hinge loss).

Computes mean(relu(topk_smallest_dist(x_pred, x_target, k) - 1.0)).

Strategy (8 NeuronCores, SPMD, x_pred rows sharded):
  - Host sorts targets by ||b||^2 and lays them out so each of 1024
    "fold chunks" (strided positions {j + 1024k}) holds 16 targets of
    nearly equal ||b||^2. Host pre-transposes to [dim, n] (factor 2 of the
    cross term folded into a) and quantizes both sides to fp8e4m3.
  - Device per core: TensorE computes 2*a.b with fp8 DoubleRow matmuls
    (issues every ~216ns for 512 targets vs ~427ns for bf16; K=128 is
    packed as 2 k-tiles with the second a-tile zeroed and the b operand
    stride-0 broadcast). PSUM is divided into four [128,1024] tiles so
    each consumer always has a prefilled tile waiting (the two-buffer
    2048-wide variant stalls every op on a refill). Per 128-row tile
    the 16 PSUM lanes are drained by ScalarE (9 wide fp32->fp16 copies)
    and DVE (7 mixed max(slab,PSUM) ops chained through 2 scratch
    lanes, seeded from the first ScalarE slab, plus one 2048-wide
    all-fp16 2x-mode fold), leaving 7168 fp16 values per row that DMA
    to the host.
  - Host finishes the fold tree (7168 -> 1024 chunk maxima), adds the
    per-chunk -min||b||^2, picks the top-32 chunks per row (fp8
    screening noise puts a true top-5 chunk at worst rank ~12 on this
    distribution; 32 leaves a wide margin), rescores the 32*16 = 512
    candidates exactly in float64, takes top-k, hinges, averages.
"""

import numpy as np

N_CORES = 8
N_PRED = 8192
N_TGT = 16384
DIM = 128
ROWS_PER_CORE = N_PRED // N_CORES  # 1024
ROWTILES = ROWS_PER_CORE // 128    # 8
LANE = 1024                        # targets per PSUM tile (2 fp32 banks)
N_LANES = N_TGT // LANE            # 16 PSUM lanes per rowtile
OUT_W = 8192                       # fp16 values per row shipped to host
FOLD_TO = 1024                     # chunk count (final, after host fold)
FOLD_S = N_TGT // FOLD_TO          # 16 targets per fold chunk
TOP_CHUNKS = 32
HINGE = 1.0

_CACHE = {}


def _build_nc():
    import concourse.bacc as bacc
    import concourse.bass as bass
    import concourse.mybir as mybir
    import concourse.tile as tile

    dt = mybir.dt
    nc = bacc.Bacc(
        "TRN2",
        target_bir_lowering=False,
        debug=False,
        num_devices=N_CORES,
    )
    # a_t: [dim, 2 k-tiles, rows]; k-tile 1 is zeros (DoubleRow packing).
    a_t = nc.dram_tensor("a_t", [DIM, 2, ROWS_PER_CORE], dt.float8e4, kind="ExternalInput")
    b_t = nc.dram_tensor("b_t", [DIM, N_TGT], dt.float8e4, kind="ExternalInput")
    cmx = nc.dram_tensor(
        "cmx", [ROWTILES, 128, OUT_W], dt.float16, kind="ExternalOutput"
    )

    DR = mybir.MatmulPerfMode.DoubleRow

    # Slab layout per rowtile (fp16, per-partition offsets in elements):
    #   s0..s7   at [0 : 8192)     ScalarE copies (s0 seeds the DVE chain)
    #   t_ping   at [8192 : 9216)  DVE chain scratch
    #   t_pong   at [9216 : 10240) DVE chain scratch (chain of 8 ends here)
    # Out pieces: s1..s7 (7168) + chain (1024) = 8192.
    SLAB_W = 10240
    T_PING, T_PONG = 8192, 9216

    with tile.TileContext(nc) as tc:
        with (
            tc.tile_pool(name="const", bufs=1) as cpool,
            tc.tile_pool(name="psum", bufs=4, space="PSUM") as ppool,
            tc.tile_pool(name="slab", bufs=3) as spool,
        ):
            bt_sb = cpool.tile([DIM, N_TGT], dt.float8e4)
            at_sb = cpool.tile([DIM, 2, ROWS_PER_CORE], dt.float8e4)

            nc.sync.dma_start(out=at_sb[:], in_=a_t[:])
            # Fine-grained slices so the first matmuls start early.
            for s in range(N_LANES):
                sl = bass.ts(s, LANE)
                nc.sync.dma_start(out=bt_sb[:, sl], in_=b_t[:, sl])

            for rt in range(ROWTILES):
                lhsT = at_sb[:, :, bass.ts(rt, 128)]  # [128, 2, 128]
                slab = spool.tile([128, SLAB_W], dt.float16)

                def mk_psum(lane, lhsT=lhsT):
                    ps = ppool.tile([128, LANE], dt.float32)
                    for j in range(LANE // 512):
                        rhs = bt_sb[:, bass.ts(lane * (LANE // 512) + j, 512)]
                        nc.tensor.matmul(
                            ps[:, bass.ts(j, 512)],
                            lhsT,
                            rhs.unsqueeze(1).broadcast_to([DIM, 2, 512]),
                            start=True,
                            stop=True,
                            perf_mode=DR,
                        )
                    return ps

                # Even lanes -> ScalarE slabs s0..s7; odd lanes -> DVE chain
                # (seeded from s0).
                n_s = 0
                n_d = 0
                chain = None
                for lane in range(N_LANES):
                    if lane % 2 == 0:
                        dst = slab[:, n_s * LANE : (n_s + 1) * LANE]
                        nc.scalar.copy(dst, mk_psum(lane)[:])
                        n_s += 1
                    else:
                        src = slab[:, 0:LANE] if chain is None else chain
                        # alternate scratch lanes to avoid in-place ops
                        dst_off = T_PING if n_d % 2 == 0 else T_PONG
                        nxt = slab[:, dst_off : dst_off + LANE]
                        nc.vector.tensor_max(nxt, src, mk_psum(lane)[:])
                        chain = nxt
                        n_d += 1
                nc.sync.dma_start(
                    out=cmx[rt][:, 0:7168], in_=slab[:, LANE : 8 * LANE]
                )
                nc.sync.dma_start(out=cmx[rt][:, 7168:8192], in_=chain)

    nc.compile()
    return nc


def _get_nc():
    if "nc" not in _CACHE:
        _CACHE["nc"] = _build_nc()
    return _CACHE["nc"]


def _prep(x_pred, x_target):
    """Host-side layout: sort targets by b2, stride into fold chunks."""
    import ml_dtypes

    b2 = np.einsum("ij,ij->i", x_target.astype(np.float64), x_target.astype(np.float64))
    order = np.argsort(b2, kind="stable")
    # position j + 1024*k holds the target of sorted rank 16*j + k
    perm = np.empty(N_TGT, np.int64)
    jj, kk = np.meshgrid(np.arange(FOLD_TO), np.arange(FOLD_S), indexing="ij")
    perm[jj + FOLD_TO * kk] = order[FOLD_S * jj + kk]

    a_t = np.zeros((DIM, 2, N_PRED), ml_dtypes.float8_e4m3)
    a_t[:, 0, :] = (2.0 * x_pred.T).astype(ml_dtypes.float8_e4m3)
    b_t = np.ascontiguousarray(x_target[perm].T.astype(ml_dtypes.float8_e4m3))
    nb2c_row = (-b2[order[::FOLD_S]]).astype(np.float32)  # -min b2 per chunk
    cand_map = order.reshape(FOLD_TO, FOLD_S)  # chunk j -> target ids
    return a_t, b_t, nb2c_row, cand_map


def _host_finish(x_pred, x_target, f1, nb2c_row, cand_map, k):
    """f1: [N_PRED, OUT_W] fp32 slab maxima; fold to chunk maxima, screen,
    rescore the top chunks exactly in float64."""
    n = x_pred.shape[0]
    f1 = f1.reshape(n, OUT_W // FOLD_TO, FOLD_TO).max(axis=1)
    chunk_val = f1 + nb2c_row
    ch = np.argpartition(-chunk_val, TOP_CHUNKS, axis=1)[:, :TOP_CHUNKS]
    tid = cand_map[ch].reshape(n, TOP_CHUNKS * FOLD_S)

    a64 = x_pred.astype(np.float64)
    b64 = x_target.astype(np.float64)
    a2 = np.einsum("ij,ij->i", a64, a64)
    b2 = np.einsum("ij,ij->i", b64, b64)

    vals = np.empty((n, k))
    B = 1024
    for s in range(0, n, B):
        t = tid[s : s + B]
        bg = b64[t]  # [B, C, DIM]
        dots = np.einsum("rd,rcd->rc", a64[s : s + B], bg, optimize=True)
        d2 = a2[s : s + B, None] + b2[t] - 2.0 * dots
        vals[s : s + B] = np.partition(d2, k - 1, axis=1)[:, :k]
    d = np.sqrt(np.maximum(vals, 0.0))
    return np.float32(np.maximum(d - HINGE, 0.0).mean(dtype=np.float64))


def _host_exact(x_pred, x_target, k):
    """Exact fallback (never expected in practice)."""
    a = x_pred.astype(np.float32)
    b = x_target.astype(np.float32)
    a2 = np.sum(a * a, axis=1)[:, None]
    b2 = np.sum(b * b, axis=1)[None, :]
    out = np.empty((a.shape[0], k), np.float64)
    B = 1024
    for s in range(0, a.shape[0], B):
        d2 = a2[s : s + B] + b2 - 2.0 * (a[s : s + B] @ b.T)
        out[s : s + B] = np.partition(d2, k - 1, axis=1)[:, :k].astype(np.float64)
    d = np.sqrt(np.maximum(out, 0.0))
    return np.float32(np.maximum(d - HINGE, 0.0).mean(dtype=np.float64))


def kernel(x_pred, x_target, top_k=5, _want_results=False):
    from concourse.bass_utils import run_bass_kernel_spmd

    x_pred = np.asarray(x_pred, dtype=np.float32)
    x_target = np.asarray(x_target, dtype=np.float32)
    k = int(top_k)
    if (
        k > TOP_CHUNKS
        or x_pred.shape != (N_PRED, DIM)
        or x_target.shape != (N_TGT, DIM)
    ):
        return _host_exact(x_pred, x_target, k)

    nc = _get_nc()
    a_t_full, b_t, nb2c_row, cand_map = _prep(x_pred, x_target)

    in_maps = []
    for c in range(N_CORES):
        in_maps.append(
            {
                "a_t": np.ascontiguousarray(
                    a_t_full[:, :, c * ROWS_PER_CORE : (c + 1) * ROWS_PER_CORE]
                ),
                "b_t": b_t,
            }
        )

    res = run_bass_kernel_spmd(nc, in_maps, list(range(N_CORES)))
    f1 = np.concatenate(
        [
            res.results[c]["cmx"].reshape(ROWS_PER_CORE, OUT_W)
            for c in range(N_CORES)
        ],
        axis=0,
    ).astype(np.float32)
    out = _host_finish(x_pred, x_target, f1, nb2c_row, cand_map, k)
    if _want_results:
        return out, res
    return out
